# revision 6
# baseline (speedup 1.0000x reference)
"""Trainium2 Bass kernel for CausalWaveletFieldAttention (v2).

Shapes (hardcoded): x [B=4, N=4096, D=1024], H=16 heads, HD=64.
Sharding over 8 cores: core c handles (batch b = c//2, half = c%2), i.e.
2048 contiguous sequence rows of one batch.

Key ideas vs v1:
  * Permuted channel layout (d_sub, head): channel c_new = d*16 + h.
    Every 128-partition chunk then contains all 16 heads for 8 d-values,
    so the HxH head coupling becomes a block-diagonal [128,128] matmul
    -- and is FUSED into the PE conv-tap stationaries (coup @ diag(w_s))
    for free. The dense 1024x1024 coupling matmul is gone.
  * Conv computes only SEQ outputs (no 1024-col halo recompute); the
    skip taps' history (coupled field tail) comes from the pair peer via
    a second small AllGather.
  * Out projection in transposed layout (Wout chunks stationary, pg
    moving, output [D, SEQ]); host transposes when assembling. No pg
    DRAM round-trip, no rearrange DMAs.
  * bf16 for x / Wk / Wv / Wgate / Wout / gate / pg (PE rate unchanged,
    halves DMA + SBUF). f0 / conv accumulation stay fp32.
  * Conv taps split across PE (3 big shifts, coupling fused) and DVE
    (21 shifts as scalar_tensor_tensor chain); skips + masks + gate-mul
    on GpSimd. Gate projection strips interleaved into the conv phase
    to fill the PE.
"""

import numpy as np
import ml_dtypes

import concourse.bass as bass
import concourse.mybir as mybir
import concourse.tile as tile
from concourse import bacc
from concourse.bass_utils import run_bass_kernel_spmd

F32 = mybir.dt.float32
F32R = mybir.dt.float32r
BF16 = mybir.dt.bfloat16
AF = mybir.ActivationFunctionType
MUL = mybir.AluOpType.mult
ADD = mybir.AluOpType.add

B, N, D, H, HD = 4, 4096, 1024, 16, 64
NCORES = 8
SEQ = N // 2          # 2048 rows per core
KC = D // 128         # 8 contraction chunks
EXT2 = 2 * SEQ        # [peer 2048 | own 2048] conv input window
D4 = [0.4829629131445341, 0.8365163037378079, 0.2241438680420134, -0.1294095225512604]
N_SCALES = 11
SPARSE_DILATIONS = (512, 1024)
SHIFTS = [0, 1, 2, 3, 4, 6, 8, 12, 16, 24, 32, 48, 64, 96, 128, 192, 256,
          384, 512, 768, 1024, 1536, 2048, 3072]
PE_TAPS = [1024, 2048, 3072]
DVE_TAPS = [s for s in SHIFTS if s not in PE_TAPS]
NPE = len(PE_TAPS)
NDVE = len(DVE_TAPS)

_PROGRAM_CACHE = {}


def _build_program(debug_outputs=False):
    key = bool(debug_outputs)
    if key in _PROGRAM_CACHE:
        return _PROGRAM_CACHE[key]

    nc = bacc.Bacc("TRN2", target_bir_lowering=False, debug=False,
                   num_devices=NCORES)

    # ---- parameters (per-core) ----
    xT = nc.declare_dram_parameter("xT", [D, SEQ], BF16, isOutput=False)
    Wkv = nc.declare_dram_parameter("Wkv", [D, 2 * D], BF16, isOutput=False)
    Wg = nc.declare_dram_parameter("Wg", [D, D], BF16, isOutput=False)
    Wo = nc.declare_dram_parameter("Wo", [D, D], BF16, isOutput=False)
    bkvT = nc.declare_dram_parameter("bkvT", [128, 16], F32, isOutput=False)
    bgT = nc.declare_dram_parameter("bgT", [128, 8], F32, isOutput=False)
    boutT = nc.declare_dram_parameter("boutT", [128, 8], F32, isOutput=False)
    BDin = nc.declare_dram_parameter("BDin", [128, NPE + 1, 128], F32R,
                                     isOutput=False)
    wtap = nc.declare_dram_parameter("wtap", [128, NDVE], F32, isOutput=False)
    swt = nc.declare_dram_parameter("swt", [128, 2], F32, isOutput=False)
    mask = nc.declare_dram_parameter("mask", [128, 1], F32, isOutput=False)
    ones16i = nc.declare_dram_parameter("ones16i", [128, 16], F32R,
                                        isOutput=False)
    on16i = nc.declare_dram_parameter("on16i", [16, 128], F32R, isOutput=False)
    outT = nc.declare_dram_parameter("outT", [D, SEQ], F32, isOutput=True)

    dbg = {}
    if debug_outputs:
        for name, shape, dt in (("dbg_f0", [D, SEQ], F32R),
                                ("dbg_field", [D, SEQ], F32),
                                ("dbg_pg", [D, SEQ], BF16)):
            dbg[name] = nc.declare_dram_parameter(name, shape, dt, isOutput=True)

    # ---- internal DRAM ----
    f0_dram = [nc.dram_tensor(f"f0_dram{c}", [128, SEQ], F32R)
               for c in range(KC)]
    f0_gath = [nc.dram_tensor(f"f0_gath{c}", [2, 128, SEQ], F32R)
               for c in range(KC)]
    tl_dram = [nc.dram_tensor(f"tl_dram{c}", [128, 1024], F32)
               for c in range(KC)]
    tl_gath = [nc.dram_tensor(f"tl_gath{c}", [2, 128, 1024], F32)
               for c in range(KC)]

    with tile.TileContext(nc) as tc:
        with (
            tc.tile_pool(name="psp", bufs=4, space="PSUM") as psp,
            tc.tile_pool(name="const", bufs=1) as constp,
            tc.tile_pool(name="p_x", bufs=1) as p_x,
            tc.tile_pool(name="p_gate", bufs=1) as p_gate,
            tc.tile_pool(name="p_pg", bufs=1) as p_pg,
            tc.tile_pool(name="p_km", bufs=1) as p_km,
            tc.tile_pool(name="p_strip", bufs=2) as p_strip,
        ):
            # ---- constants ----
            BD_t = constp.tile([128, NPE + 1, 128], F32R)
            nc.sync.dma_start(BD_t[:], BDin[:])
            wtap_t = constp.tile([128, NDVE], F32)
            nc.sync.dma_start(wtap_t[:], wtap[:])
            swt_t = constp.tile([128, 2], F32)
            nc.sync.dma_start(swt_t[:], swt[:])
            mask_t = constp.tile([128, 1], F32)
            nc.sync.dma_start(mask_t[:], mask[:])
            bkv_t = constp.tile([128, 16], F32)
            nc.sync.dma_start(bkv_t[:], bkvT[:])
            bg_t = constp.tile([128, 8], F32)
            nc.sync.dma_start(bg_t[:], bgT[:])
            bout_t = constp.tile([128, 8], F32)
            nc.sync.dma_start(bout_t[:], boutT[:])
            ones16 = constp.tile([128, 16], F32R)
            nc.sync.dma_start(ones16[:], ones16i[:])
            on16 = constp.tile([16, 128], F32R)
            nc.sync.dma_start(on16[:], on16i[:])

            # ---- persistent big tiles ----
            xm = p_x.tile([128, KC, SEQ], BF16)
            for k in range(KC):
                nc.sync.dma_start(xm[:, k, :], xT[k * 128:(k + 1) * 128, :])
            gate = p_gate.tile([128, KC, SEQ], BF16)
            pg = p_pg.tile([128, KC, SEQ], BF16)
            km_t = p_km.tile([16, SEQ], F32R)

            def load_strip(src, col0):
                sr = p_strip.tile([128, KC, 128], BF16, tag="strip")
                nc.sync.dma_start(
                    sr[:],
                    src[:, col0 * 128:(col0 + 1) * 128]
                    .rearrange("(kc p) m -> p kc m", p=128))
                return sr

            def proj4(strip, tag="ps"):
                """4 PSUM tiles [128,512] = strip.T @ xm (contraction D)."""
                pss = [psp.tile([128, 512], F32, tag=tag, name=f"pj{rb}")
                       for rb in range(4)]
                for k in range(KC):
                    for rb in range(4):
                        nc.tensor.matmul(pss[rb][:], strip[:, k, :],
                                         xm[:, k, rb * 512:(rb + 1) * 512],
                                         start=(k == 0), stop=(k == KC - 1))
                return pss

            # ================= phase A: k -> k_mag =================
            with (
                tc.tile_pool(name="pskm", bufs=1, space="PSUM") as pskm,
                tc.tile_pool(name="p_kv", bufs=2) as p_kv,
            ):
                km_pss = [pskm.tile([16, 512], F32, tag=f"km{sb}",
                                    name=f"km_ps{sb}")
                          for sb in range(4)]
                for c in range(KC):
                    ks = load_strip(Wkv, c)
                    pss = proj4(ks)
                    k2b = p_kv.tile([128, SEQ], F32R, tag="k2b")
                    for rb in range(4):
                        nc.scalar.activation(k2b[:, rb * 512:(rb + 1) * 512],
                                             pss[rb][:], AF.Square,
                                             bias=bkv_t[:, c:c + 1])
                    for sb in range(4):
                        nc.tensor.matmul(km_pss[sb][:], ones16[:],
                                         k2b[:, sb * 512:(sb + 1) * 512],
                                         start=(c == 0), stop=(c == KC - 1))
                for sb in range(4):
                    nc.scalar.activation(km_t[:, sb * 512:(sb + 1) * 512],
                                         km_pss[sb][:], AF.Sqrt)

            # ================= phase B: v -> f0 -> AllGather =================
            with (
                tc.tile_pool(name="psv", bufs=2, space="PSUM") as psv,
                tc.tile_pool(name="p_v", bufs=2) as p_v,
            ):
                for c in range(KC):
                    vs = load_strip(Wkv, 8 + c)
                    pss = proj4(vs)
                    vTb = p_v.tile([128, SEQ], F32, tag="vTb")
                    for rb in range(4):
                        nc.scalar.activation(vTb[:, rb * 512:(rb + 1) * 512],
                                             pss[rb][:], AF.Identity,
                                             bias=bkv_t[:, 8 + c:9 + c])
                    f0b = p_v.tile([128, SEQ], F32R, tag="f0b")
                    for sb in range(4):
                        bps = psv.tile([128, 512], F32, tag="bc")
                        nc.tensor.matmul(bps[:], on16[:],
                                         km_t[:, sb * 512:(sb + 1) * 512],
                                         start=True, stop=True)
                        nc.vector.tensor_mul(f0b[:, sb * 512:(sb + 1) * 512],
                                             vTb[:, sb * 512:(sb + 1) * 512],
                                             bps[:])
                    nc.sync.dma_start(f0_dram[c][:], f0b[:])
                    nc.gpsimd.collective_compute(
                        "AllGather", mybir.AluOpType.bypass,
                        replica_groups=[[0, 1], [2, 3], [4, 5], [6, 7]],
                        ins=[f0_dram[c][:]], outs=[f0_gath[c][:]])

            if debug_outputs:
                for c in range(KC):
                    nc.sync.dma_start(
                        dbg["dbg_f0"][c * 128:(c + 1) * 128, :], f0_dram[c][:])

            # ======== phase C: conv + coupling + skips + gate (interleaved) ==
            with (
                tc.tile_pool(name="p_ext", bufs=2) as p_ext,
                tc.tile_pool(name="p_acc", bufs=1) as p_acc,
                tc.tile_pool(name="p_fx", bufs=2) as p_fx,
                tc.tile_pool(name="p_sk", bufs=2) as p_sk,
            ):
                fexts = [None] * KC
                for c in range(KC + 1):
                    if c < KC:
                        # gate strip c (PE fills while DVE/GP chew on conv)
                        gs = load_strip(Wg, c)
                        pss = proj4(gs)
                        for rb in range(4):
                            nc.scalar.activation(
                                gate[:, c, rb * 512:(rb + 1) * 512],
                                pss[rb][:], AF.Sigmoid, bias=bg_t[:, c:c + 1])

                        # conv input window [peer | own]
                        ext = p_ext.tile([128, EXT2], F32R, tag="ext")
                        nc.sync.dma_start(ext[:, 0:SEQ], f0_gath[c][0, :, :])
                        nc.scalar.activation(ext[:, 0:SEQ], ext[:, 0:SEQ],
                                             AF.Identity,
                                             scale=mask_t[:, 0:1])
                        nc.sync.dma_start(ext[:, SEQ:EXT2], f0_dram[c][:])

                        # DVE tap chain (uncoupled accumulation)
                        da0 = p_acc.tile([128, SEQ], F32R, tag="da0")
                        da1 = p_acc.tile([128, SEQ], F32R, tag="da1")
                        da = [da0, da1]
                        cur = None
                        for ti, s in enumerate(DVE_TAPS):
                            src = ext[:, SEQ - s:EXT2 - s]
                            w = wtap_t[:, ti:ti + 1]
                            if cur is None:
                                cur = da[0]
                                nc.vector.tensor_scalar_mul(cur[:], src, w)
                            else:
                                nxt = da[ti % 2]
                                nc.vector.scalar_tensor_tensor(
                                    nxt[:], src, w, cur[:], op0=MUL, op1=ADD)
                                cur = nxt

                        # PE taps (coupling fused) + coupled DVE acc -> field
                        fext = p_fx.tile([128, 1024 + SEQ], F32, tag="fext")
                        fexts[c] = fext
                        for ob in range(4):
                            ps = psp.tile([128, 512], F32, tag="ps")
                            started = False
                            for ti, s in enumerate(PE_TAPS):
                                e0 = SEQ + ob * 512 - s
                                if e0 + 512 <= 0:
                                    continue
                                nc.tensor.matmul(ps[:], BD_t[:, ti, :],
                                                 ext[:, e0:e0 + 512],
                                                 start=not started, stop=False)
                                started = True
                            nc.tensor.matmul(ps[:], BD_t[:, NPE, :],
                                             cur[:, ob * 512:(ob + 1) * 512],
                                             start=not started, stop=True)
                            nc.scalar.activation(
                                fext[:, 1024 + ob * 512:1024 + (ob + 1) * 512],
                                ps[:], AF.Identity)

                        # coupled-field tail -> peer
                        nc.sync.dma_start(tl_dram[c][:], fext[:, SEQ:SEQ + 1024])
                        nc.gpsimd.collective_compute(
                            "AllGather", mybir.AluOpType.bypass,
                            replica_groups=[[0, 1], [2, 3], [4, 5], [6, 7]],
                            ins=[tl_dram[c][:]], outs=[tl_gath[c][:]])

                    if c >= 1:
                        # skip taps + gate multiply for chunk c-1 (peer tail
                        # has had a full chunk-time to arrive)
                        cp = c - 1
                        fext = fexts[cp]
                        nc.sync.dma_start(fext[:, 0:1024], tl_gath[cp][0, :, :])
                        nc.scalar.activation(fext[:, 0:1024], fext[:, 0:1024],
                                             AF.Identity,
                                             scale=mask_t[:, 0:1])
                        tmpb = p_sk.tile([128, SEQ], F32, tag="tmpb")
                        nc.vector.scalar_tensor_tensor(
                            tmpb[:], fext[:, 512:512 + SEQ], swt_t[:, 0:1],
                            fext[:, 1024:1024 + SEQ], op0=MUL, op1=ADD)
                        ub = p_sk.tile([128, SEQ], BF16, tag="ub")
                        nc.vector.scalar_tensor_tensor(
                            ub[:], fext[:, 0:SEQ], swt_t[:, 1:2],
                            tmpb[:], op0=MUL, op1=ADD)
                        nc.gpsimd.tensor_mul(pg[:, cp, :], ub[:],
                                             gate[:, cp, :])
                        if debug_outputs:
                            nc.sync.dma_start(
                                dbg["dbg_field"][cp * 128:(cp + 1) * 128, :],
                                fext[:, 1024:1024 + SEQ])
                            nc.sync.dma_start(
                                dbg["dbg_pg"][cp * 128:(cp + 1) * 128, :],
                                pg[:, cp, :])

            # ================= phase D: outT = Wo.T-chunks @ pg =============
            with tc.tile_pool(name="p_out", bufs=2) as p_out:
                for co in range(KC):
                    wos = load_strip(Wo, co)
                    pss = [psp.tile([128, 512], F32, tag="ps",
                                    name=f"ops{sb}")
                           for sb in range(4)]
                    for k in range(KC):
                        for sb in range(4):
                            nc.tensor.matmul(pss[sb][:], wos[:, k, :],
                                             pg[:, k, sb * 512:(sb + 1) * 512],
                                             start=(k == 0), stop=(k == KC - 1))
                    outb = p_out.tile([128, SEQ], F32, tag="outb")
                    for sb in range(4):
                        nc.scalar.activation(outb[:, sb * 512:(sb + 1) * 512],
                                             pss[sb][:], AF.Identity,
                                             bias=bout_t[:, co:co + 1])
                    nc.sync.dma_start(outT[co * 128:(co + 1) * 128, :],
                                      outb[:])

    nc.compile()
    _PROGRAM_CACHE[key] = nc
    return nc


def _softmax(a, axis):
    a = a - a.max(axis=axis, keepdims=True)
    e = np.exp(a)
    return e / e.sum(axis=axis, keepdims=True)


def _blockdiag8(blk16):
    m = np.zeros((128, 128), np.float64)
    for d8 in range(8):
        m[d8 * 16:(d8 + 1) * 16, d8 * 16:(d8 + 1) * 16] = blk16
    return m


def _host_prep(inputs):
    """Build per-core and replicated input tensors from full inputs."""
    x = np.asarray(inputs["x"], np.float32)
    Wqkv = np.asarray(inputs["Wqkv"], np.float64)
    bqkv = np.asarray(inputs["bqkv"], np.float64)
    Wout = np.asarray(inputs["Wout"], np.float64)
    bout = np.asarray(inputs["bout"], np.float32)
    Wgate = np.asarray(inputs["Wgate"], np.float64)
    bgate = np.asarray(inputs["bgate"], np.float64)
    scale_gain = np.asarray(inputs["scale_gain"], np.float64)
    skip_w = np.asarray(inputs["skip_w"], np.float64)
    coupling = np.asarray(inputs["coupling"], np.float64)

    gains = _softmax(scale_gain, axis=0)              # [11, H]
    sw = 1.0 / (1.0 + np.exp(-skip_w))                # [2]
    coup = _softmax(coupling, axis=-1)                # [H, H]

    # channel permutation: new channel c_new = d*16 + h  <->  ref h*64 + d
    cn = np.arange(D)
    perm_idx = (cn % H) * HD + cn // H                # ref index per new chan

    sidx = {s: i for i, s in enumerate(SHIFTS)}
    wtab = np.zeros((len(SHIFTS), H), np.float64)
    for j in range(N_SCALES):
        d = 1 << j
        for t in range(4):
            wtab[sidx[(3 - t) * d]] += D4[t] * gains[j]

    # PE tap stationaries: block[j, i] = coup[i, j] * w_s[j]; + pure coupling
    BDt = np.zeros((128, NPE + 1, 128), np.float32)
    for ti, s in enumerate(PE_TAPS):
        M = coup * wtab[sidx[s]][None, :]             # M[i, j]
        BDt[:, ti, :] = _blockdiag8(M.T).astype(np.float32)
    BDt[:, NPE, :] = _blockdiag8(coup.T).astype(np.float32)

    wtap_np = np.zeros((128, NDVE), np.float32)
    for ti, s in enumerate(DVE_TAPS):
        wtap_np[:, ti] = wtab[sidx[s]][np.arange(128) % H]

    ones16 = np.zeros((128, 16), np.float32)
    ones16[np.arange(128), np.arange(128) % 16] = 1.0
    on16 = np.ascontiguousarray(ones16.T)             # [16, 128]

    # fold q projection into the gate
    Wq = Wqkv[:, :D]
    Wqg = Wq @ Wgate
    bg_f = bqkv[:D] @ Wgate + bgate

    bf = ml_dtypes.bfloat16
    Wk_p = Wqkv[:, D:2 * D][:, perm_idx]
    Wv_p = Wqkv[:, 2 * D:3 * D][:, perm_idx]
    Wkv_np = np.ascontiguousarray(
        np.concatenate([Wk_p, Wv_p], axis=1).astype(bf))
    Wg_np = np.ascontiguousarray(Wqg[:, perm_idx].astype(bf))
    Wo_np = np.ascontiguousarray(Wout[perm_idx, :].astype(bf))

    bk_p = bqkv[D:2 * D][perm_idx].astype(np.float32)
    bv_p = bqkv[2 * D:3 * D][perm_idx].astype(np.float32)
    bkvT_np = np.concatenate([bk_p.reshape(8, 128).T,
                              bv_p.reshape(8, 128).T], axis=1).copy()
    bgT_np = bg_f[perm_idx].astype(np.float32).reshape(8, 128).T.copy()
    boutT_np = bout.reshape(8, 128).T.copy()
    swt_np = np.broadcast_to(sw.astype(np.float32), (128, 2)).copy()

    shared = dict(Wkv=Wkv_np, Wg=Wg_np, Wo=Wo_np, bkvT=bkvT_np, bgT=bgT_np,
                  boutT=boutT_np, BDin=BDt, wtap=wtap_np, swt=swt_np,
                  ones16i=ones16, on16i=on16)
    in_maps = []
    for c in range(NCORES):
        b, half = c // 2, c % 2
        g0 = half * SEQ
        xTc = np.ascontiguousarray(x[b, g0:g0 + SEQ, :].T.astype(bf))
        m = np.full((128, 1), float(half), np.float32)
        in_maps.append(dict(xT=xTc, mask=m, **shared))
    return in_maps


def run_cores(inputs, debug_outputs=False, trace=False):
    nc = _build_program(debug_outputs=debug_outputs)
    in_maps = _host_prep(inputs)
    res = run_bass_kernel_spmd(nc, in_maps, list(range(NCORES)), trace=trace)
    return res


def kernel(**inputs) -> np.ndarray:
    res = run_cores(inputs)
    out = np.empty((B, N, D), np.float32)
    for c in range(NCORES):
        b, half = c // 2, c % 2
        out[b, half * SEQ:(half + 1) * SEQ, :] = res.results[c]["outT"].T
    return out


# revision 8
# speedup vs baseline: 1.0064x; 1.0064x over previous
"""Trainium2 Bass kernel for CausalWaveletFieldAttention (v2).

Shapes (hardcoded): x [B=4, N=4096, D=1024], H=16 heads, HD=64.
Sharding over 8 cores: core c handles (batch b = c//2, half = c%2), i.e.
2048 contiguous sequence rows of one batch.

Key ideas vs v1:
  * Permuted channel layout (d_sub, head): channel c_new = d*16 + h.
    Every 128-partition chunk then contains all 16 heads for 8 d-values,
    so the HxH head coupling becomes a block-diagonal [128,128] matmul
    -- and is FUSED into the PE conv-tap stationaries (coup @ diag(w_s))
    for free. The dense 1024x1024 coupling matmul is gone.
  * Conv computes only SEQ outputs (no 1024-col halo recompute); the
    skip taps' history (coupled field tail) comes from the pair peer via
    a second small AllGather.
  * Out projection in transposed layout (Wout chunks stationary, pg
    moving, output [D, SEQ]); host transposes when assembling. No pg
    DRAM round-trip, no rearrange DMAs.
  * bf16 for x / Wk / Wv / Wgate / Wout / gate / pg (PE rate unchanged,
    halves DMA + SBUF). f0 / conv accumulation stay fp32.
  * Conv taps split across PE (3 big shifts, coupling fused) and DVE
    (21 shifts as scalar_tensor_tensor chain); skips + masks + gate-mul
    on GpSimd. Gate projection strips interleaved into the conv phase
    to fill the PE.
"""

import numpy as np
import ml_dtypes

import concourse.bass as bass
import concourse.mybir as mybir
import concourse.tile as tile
from concourse import bacc
from concourse.bass_utils import run_bass_kernel_spmd

F32 = mybir.dt.float32
F32R = mybir.dt.float32r
BF16 = mybir.dt.bfloat16
AF = mybir.ActivationFunctionType
MUL = mybir.AluOpType.mult
ADD = mybir.AluOpType.add

B, N, D, H, HD = 4, 4096, 1024, 16, 64
NCORES = 8
SEQ = N // 2          # 2048 rows per core
KC = D // 128         # 8 contraction chunks
EXT2 = 2 * SEQ        # [peer 2048 | own 2048] conv input window
D4 = [0.4829629131445341, 0.8365163037378079, 0.2241438680420134, -0.1294095225512604]
N_SCALES = 11
SPARSE_DILATIONS = (512, 1024)
SHIFTS = [0, 1, 2, 3, 4, 6, 8, 12, 16, 24, 32, 48, 64, 96, 128, 192, 256,
          384, 512, 768, 1024, 1536, 2048, 3072]
PE_TAPS = [1024, 2048, 3072]
DVE_TAPS = [s for s in SHIFTS if s not in PE_TAPS]
NPE = len(PE_TAPS)
NDVE = len(DVE_TAPS)

_PROGRAM_CACHE = {}


def _build_program(debug_outputs=False):
    key = bool(debug_outputs)
    if key in _PROGRAM_CACHE:
        return _PROGRAM_CACHE[key]

    nc = bacc.Bacc("TRN2", target_bir_lowering=False, debug=False,
                   num_devices=NCORES)

    # ---- parameters (per-core) ----
    xT = nc.declare_dram_parameter("xT", [D, SEQ], BF16, isOutput=False)
    Wkv = nc.declare_dram_parameter("Wkv", [D, 2 * D], BF16, isOutput=False)
    Wg = nc.declare_dram_parameter("Wg", [D, D], BF16, isOutput=False)
    Wo = nc.declare_dram_parameter("Wo", [D, D], BF16, isOutput=False)
    bkvT = nc.declare_dram_parameter("bkvT", [128, 16], F32, isOutput=False)
    bgT = nc.declare_dram_parameter("bgT", [128, 8], F32, isOutput=False)
    boutT = nc.declare_dram_parameter("boutT", [128, 8], F32, isOutput=False)
    BDin = nc.declare_dram_parameter("BDin", [128, NPE + 1, 128], F32R,
                                     isOutput=False)
    wtap = nc.declare_dram_parameter("wtap", [128, NDVE], F32, isOutput=False)
    swt = nc.declare_dram_parameter("swt", [128, 2], F32, isOutput=False)
    mask = nc.declare_dram_parameter("mask", [128, 1], F32, isOutput=False)
    ones16i = nc.declare_dram_parameter("ones16i", [128, 16], F32R,
                                        isOutput=False)
    on16i = nc.declare_dram_parameter("on16i", [16, 128], F32R, isOutput=False)
    outT = nc.declare_dram_parameter("outT", [D, SEQ], F32, isOutput=True)

    dbg = {}
    if debug_outputs:
        for name, shape, dt in (("dbg_f0", [D, SEQ], F32R),
                                ("dbg_field", [D, SEQ], F32),
                                ("dbg_pg", [D, SEQ], BF16)):
            dbg[name] = nc.declare_dram_parameter(name, shape, dt, isOutput=True)

    # ---- internal DRAM ----
    f0_dram = [nc.dram_tensor(f"f0_dram{c}", [128, SEQ], F32R)
               for c in range(KC)]
    f0_gath = [nc.dram_tensor(f"f0_gath{c}", [2, 128, SEQ], F32R)
               for c in range(KC)]
    tl_dram = [nc.dram_tensor(f"tl_dram{c}", [128, 1024], F32)
               for c in range(KC)]
    tl_gath = [nc.dram_tensor(f"tl_gath{c}", [2, 128, 1024], F32)
               for c in range(KC)]

    with tile.TileContext(nc) as tc:
        with (
            tc.tile_pool(name="psp", bufs=4, space="PSUM") as psp,
            tc.tile_pool(name="const", bufs=1) as constp,
            tc.tile_pool(name="p_x", bufs=1) as p_x,
            tc.tile_pool(name="p_gate", bufs=1) as p_gate,
            tc.tile_pool(name="p_pg", bufs=1) as p_pg,
            tc.tile_pool(name="p_strip", bufs=2) as p_strip,
        ):
            # ---- constants ----
            BD_t = constp.tile([128, NPE + 1, 128], F32R)
            nc.sync.dma_start(BD_t[:], BDin[:])
            wtap_t = constp.tile([128, NDVE], F32)
            nc.sync.dma_start(wtap_t[:], wtap[:])
            swt_t = constp.tile([128, 2], F32)
            nc.sync.dma_start(swt_t[:], swt[:])
            mask_t = constp.tile([128, 1], F32)
            nc.sync.dma_start(mask_t[:], mask[:])
            bkv_t = constp.tile([128, 16], F32)
            nc.sync.dma_start(bkv_t[:], bkvT[:])
            bg_t = constp.tile([128, 8], F32)
            nc.sync.dma_start(bg_t[:], bgT[:])
            bout_t = constp.tile([128, 8], F32)
            nc.sync.dma_start(bout_t[:], boutT[:])
            ones16 = constp.tile([128, 16], F32R)
            nc.sync.dma_start(ones16[:], ones16i[:])
            on16 = constp.tile([16, 128], F32R)
            nc.sync.dma_start(on16[:], on16i[:])

            # ---- persistent big tiles ----
            xm = p_x.tile([128, KC, SEQ], BF16)
            for k in range(KC):
                nc.sync.dma_start(xm[:, k, :], xT[k * 128:(k + 1) * 128, :])
            gate = p_gate.tile([128, KC, SEQ], BF16)
            pg = p_pg.tile([128, KC, SEQ], BF16)

            def load_strip(src, col0):
                sr = p_strip.tile([128, KC, 128], BF16, tag="strip")
                nc.sync.dma_start(
                    sr[:],
                    src[:, col0 * 128:(col0 + 1) * 128]
                    .rearrange("(kc p) m -> p kc m", p=128))
                return sr

            def proj4(strip, tag="ps"):
                """4 PSUM tiles [128,512] = strip.T @ xm (contraction D)."""
                pss = [psp.tile([128, 512], F32, tag=tag, name=f"pj{rb}")
                       for rb in range(4)]
                for k in range(KC):
                    for rb in range(4):
                        nc.tensor.matmul(pss[rb][:], strip[:, k, :],
                                         xm[:, k, rb * 512:(rb + 1) * 512],
                                         start=(k == 0), stop=(k == KC - 1))
                return pss

            # ================= phase A: k -> k_mag =================
            with tc.tile_pool(name="p_km", bufs=1) as p_km:
              km_t = p_km.tile([16, SEQ], F32R)
              with (
                tc.tile_pool(name="pskm", bufs=1, space="PSUM") as pskm,
                tc.tile_pool(name="p_kv", bufs=2) as p_kv,
              ):
                km_pss = [pskm.tile([16, 512], F32, tag=f"km{sb}",
                                    name=f"km_ps{sb}")
                          for sb in range(4)]
                for c in range(KC):
                    ks = load_strip(Wkv, c)
                    pss = proj4(ks)
                    k2b = p_kv.tile([128, SEQ], F32R, tag="k2b")
                    for rb in range(4):
                        nc.scalar.activation(k2b[:, rb * 512:(rb + 1) * 512],
                                             pss[rb][:], AF.Square,
                                             bias=bkv_t[:, c:c + 1])
                    for sb in range(4):
                        nc.tensor.matmul(km_pss[sb][:], ones16[:],
                                         k2b[:, sb * 512:(sb + 1) * 512],
                                         start=(c == 0), stop=(c == KC - 1))
                for sb in range(4):
                    nc.scalar.activation(km_t[:, sb * 512:(sb + 1) * 512],
                                         km_pss[sb][:], AF.Sqrt)

              # =============== phase B: v -> f0 -> AllGather ===============
              with (
                tc.tile_pool(name="psv", bufs=2, space="PSUM") as psv,
                tc.tile_pool(name="p_v", bufs=2) as p_v,
              ):
                for c in range(KC):
                    vs = load_strip(Wkv, 8 + c)
                    pss = proj4(vs)
                    vTb = p_v.tile([128, SEQ], F32, tag="vTb")
                    for rb in range(4):
                        nc.scalar.activation(vTb[:, rb * 512:(rb + 1) * 512],
                                             pss[rb][:], AF.Identity,
                                             bias=bkv_t[:, 8 + c:9 + c])
                    f0b = p_v.tile([128, SEQ], F32R, tag="f0b")
                    for sb in range(4):
                        bps = psv.tile([128, 512], F32, tag="bc")
                        nc.tensor.matmul(bps[:], on16[:],
                                         km_t[:, sb * 512:(sb + 1) * 512],
                                         start=True, stop=True)
                        nc.vector.tensor_mul(f0b[:, sb * 512:(sb + 1) * 512],
                                             vTb[:, sb * 512:(sb + 1) * 512],
                                             bps[:])
                    nc.sync.dma_start(f0_dram[c][:], f0b[:])
                    nc.gpsimd.collective_compute(
                        "AllGather", mybir.AluOpType.bypass,
                        replica_groups=[[0, 1], [2, 3], [4, 5], [6, 7]],
                        ins=[f0_dram[c][:]], outs=[f0_gath[c][:]])

            if debug_outputs:
                for c in range(KC):
                    nc.sync.dma_start(
                        dbg["dbg_f0"][c * 128:(c + 1) * 128, :], f0_dram[c][:])

            # ======== phase C: conv + coupling + skips + gate (interleaved) ==
            with (
                tc.tile_pool(name="psc", bufs=2, space="PSUM") as psc,
                tc.tile_pool(name="p_ext", bufs=2) as p_ext,
                tc.tile_pool(name="p_acc", bufs=1) as p_acc,
                tc.tile_pool(name="p_fx", bufs=3) as p_fx,
                tc.tile_pool(name="p_sk", bufs=1) as p_sk,
            ):
                fexts = [None] * KC
                LAG = 2
                for c in range(KC + LAG):
                    if c < KC:
                        # gate strip c (PE fills while DVE/GP chew on conv)
                        gs = load_strip(Wg, c)
                        pss = proj4(gs)
                        for rb in range(4):
                            nc.scalar.activation(
                                gate[:, c, rb * 512:(rb + 1) * 512],
                                pss[rb][:], AF.Sigmoid, bias=bg_t[:, c:c + 1])

                        # conv input window [peer | own]
                        ext = p_ext.tile([128, EXT2], F32R, tag="ext")
                        nc.sync.dma_start(ext[:, 0:SEQ], f0_gath[c][0, :, :])
                        nc.scalar.activation(ext[:, 0:SEQ], ext[:, 0:SEQ],
                                             AF.Identity,
                                             scale=mask_t[:, 0:1])
                        nc.sync.dma_start(ext[:, SEQ:EXT2], f0_dram[c][:])

                        # DVE tap chain (uncoupled accumulation)
                        da0 = p_acc.tile([128, SEQ], F32R, tag="da0", bufs=2)
                        da1 = p_acc.tile([128, SEQ], F32R, tag="da1", bufs=1)
                        da = [da0, da1]
                        cur = None
                        for ti, s in enumerate(DVE_TAPS):
                            src = ext[:, SEQ - s:EXT2 - s]
                            w = wtap_t[:, ti:ti + 1]
                            if cur is None:
                                cur = da[0]
                                nc.vector.tensor_scalar_mul(cur[:], src, w)
                            else:
                                nxt = da[ti % 2]
                                nc.vector.scalar_tensor_tensor(
                                    nxt[:], src, w, cur[:], op0=MUL, op1=ADD)
                                cur = nxt

                        # PE taps (coupling fused) + coupled DVE acc -> field
                        fext = p_fx.tile([128, 1024 + SEQ], F32, tag="fext")
                        fexts[c] = fext
                        for ob in range(4):
                            ps = psc.tile([128, 512], F32, tag="cps")
                            started = False
                            for ti, s in enumerate(PE_TAPS):
                                e0 = SEQ + ob * 512 - s
                                if e0 + 512 <= 0:
                                    continue
                                nc.tensor.matmul(ps[:], BD_t[:, ti, :],
                                                 ext[:, e0:e0 + 512],
                                                 start=not started, stop=False)
                                started = True
                            nc.tensor.matmul(ps[:], BD_t[:, NPE, :],
                                             cur[:, ob * 512:(ob + 1) * 512],
                                             start=not started, stop=True)
                            nc.scalar.activation(
                                fext[:, 1024 + ob * 512:1024 + (ob + 1) * 512],
                                ps[:], AF.Identity)

                        # coupled-field tail -> peer
                        nc.sync.dma_start(tl_dram[c][:], fext[:, SEQ:SEQ + 1024])
                        nc.gpsimd.collective_compute(
                            "AllGather", mybir.AluOpType.bypass,
                            replica_groups=[[0, 1], [2, 3], [4, 5], [6, 7]],
                            ins=[tl_dram[c][:]], outs=[tl_gath[c][:]])

                    if c >= LAG:
                        # skip taps + gate multiply for chunk c-LAG (peer tail
                        # has had two chunk-times to arrive)
                        cp = c - LAG
                        fext = fexts[cp]
                        nc.sync.dma_start(fext[:, 0:1024], tl_gath[cp][0, :, :])
                        nc.scalar.activation(fext[:, 0:1024], fext[:, 0:1024],
                                             AF.Identity,
                                             scale=mask_t[:, 0:1])
                        tmpb = p_sk.tile([128, SEQ], F32, tag="tmpb")
                        nc.vector.scalar_tensor_tensor(
                            tmpb[:], fext[:, 512:512 + SEQ], swt_t[:, 0:1],
                            fext[:, 1024:1024 + SEQ], op0=MUL, op1=ADD)
                        nc.vector.scalar_tensor_tensor(
                            pg[:, cp, :], fext[:, 0:SEQ], swt_t[:, 1:2],
                            tmpb[:], op0=MUL, op1=ADD)
                        nc.gpsimd.tensor_mul(pg[:, cp, :], pg[:, cp, :],
                                             gate[:, cp, :])
                        if debug_outputs:
                            nc.sync.dma_start(
                                dbg["dbg_field"][cp * 128:(cp + 1) * 128, :],
                                fext[:, 1024:1024 + SEQ])
                            nc.sync.dma_start(
                                dbg["dbg_pg"][cp * 128:(cp + 1) * 128, :],
                                pg[:, cp, :])

            # ================= phase D: outT = Wo.T-chunks @ pg =============
            with tc.tile_pool(name="p_out", bufs=2) as p_out:
                for co in range(KC):
                    wos = load_strip(Wo, co)
                    pss = [psp.tile([128, 512], F32, tag="ps",
                                    name=f"ops{sb}")
                           for sb in range(4)]
                    for k in range(KC):
                        for sb in range(4):
                            nc.tensor.matmul(pss[sb][:], wos[:, k, :],
                                             pg[:, k, sb * 512:(sb + 1) * 512],
                                             start=(k == 0), stop=(k == KC - 1))
                    outb = p_out.tile([128, SEQ], F32, tag="outb")
                    for sb in range(4):
                        nc.scalar.activation(outb[:, sb * 512:(sb + 1) * 512],
                                             pss[sb][:], AF.Identity,
                                             bias=bout_t[:, co:co + 1])
                    nc.sync.dma_start(outT[co * 128:(co + 1) * 128, :],
                                      outb[:])

    nc.compile()
    _PROGRAM_CACHE[key] = nc
    return nc


def _softmax(a, axis):
    a = a - a.max(axis=axis, keepdims=True)
    e = np.exp(a)
    return e / e.sum(axis=axis, keepdims=True)


def _blockdiag8(blk16):
    m = np.zeros((128, 128), np.float64)
    for d8 in range(8):
        m[d8 * 16:(d8 + 1) * 16, d8 * 16:(d8 + 1) * 16] = blk16
    return m


def _host_prep(inputs):
    """Build per-core and replicated input tensors from full inputs."""
    x = np.asarray(inputs["x"], np.float32)
    Wqkv = np.asarray(inputs["Wqkv"], np.float64)
    bqkv = np.asarray(inputs["bqkv"], np.float64)
    Wout = np.asarray(inputs["Wout"], np.float64)
    bout = np.asarray(inputs["bout"], np.float32)
    Wgate = np.asarray(inputs["Wgate"], np.float64)
    bgate = np.asarray(inputs["bgate"], np.float64)
    scale_gain = np.asarray(inputs["scale_gain"], np.float64)
    skip_w = np.asarray(inputs["skip_w"], np.float64)
    coupling = np.asarray(inputs["coupling"], np.float64)

    gains = _softmax(scale_gain, axis=0)              # [11, H]
    sw = 1.0 / (1.0 + np.exp(-skip_w))                # [2]
    coup = _softmax(coupling, axis=-1)                # [H, H]

    # channel permutation: new channel c_new = d*16 + h  <->  ref h*64 + d
    cn = np.arange(D)
    perm_idx = (cn % H) * HD + cn // H                # ref index per new chan

    sidx = {s: i for i, s in enumerate(SHIFTS)}
    wtab = np.zeros((len(SHIFTS), H), np.float64)
    for j in range(N_SCALES):
        d = 1 << j
        for t in range(4):
            wtab[sidx[(3 - t) * d]] += D4[t] * gains[j]

    # PE tap stationaries: block[j, i] = coup[i, j] * w_s[j]; + pure coupling
    BDt = np.zeros((128, NPE + 1, 128), np.float32)
    for ti, s in enumerate(PE_TAPS):
        M = coup * wtab[sidx[s]][None, :]             # M[i, j]
        BDt[:, ti, :] = _blockdiag8(M.T).astype(np.float32)
    BDt[:, NPE, :] = _blockdiag8(coup.T).astype(np.float32)

    wtap_np = np.zeros((128, NDVE), np.float32)
    for ti, s in enumerate(DVE_TAPS):
        wtap_np[:, ti] = wtab[sidx[s]][np.arange(128) % H]

    ones16 = np.zeros((128, 16), np.float32)
    ones16[np.arange(128), np.arange(128) % 16] = 1.0
    on16 = np.ascontiguousarray(ones16.T)             # [16, 128]

    # fold q projection into the gate
    Wq = Wqkv[:, :D]
    Wqg = Wq @ Wgate
    bg_f = bqkv[:D] @ Wgate + bgate

    bf = ml_dtypes.bfloat16
    Wk_p = Wqkv[:, D:2 * D][:, perm_idx]
    Wv_p = Wqkv[:, 2 * D:3 * D][:, perm_idx]
    Wkv_np = np.ascontiguousarray(
        np.concatenate([Wk_p, Wv_p], axis=1).astype(bf))
    Wg_np = np.ascontiguousarray(Wqg[:, perm_idx].astype(bf))
    Wo_np = np.ascontiguousarray(Wout[perm_idx, :].astype(bf))

    bk_p = bqkv[D:2 * D][perm_idx].astype(np.float32)
    bv_p = bqkv[2 * D:3 * D][perm_idx].astype(np.float32)
    bkvT_np = np.concatenate([bk_p.reshape(8, 128).T,
                              bv_p.reshape(8, 128).T], axis=1).copy()
    bgT_np = bg_f[perm_idx].astype(np.float32).reshape(8, 128).T.copy()
    boutT_np = bout.reshape(8, 128).T.copy()
    swt_np = np.broadcast_to(sw.astype(np.float32), (128, 2)).copy()

    shared = dict(Wkv=Wkv_np, Wg=Wg_np, Wo=Wo_np, bkvT=bkvT_np, bgT=bgT_np,
                  boutT=boutT_np, BDin=BDt, wtap=wtap_np, swt=swt_np,
                  ones16i=ones16, on16i=on16)
    in_maps = []
    for c in range(NCORES):
        b, half = c // 2, c % 2
        g0 = half * SEQ
        xTc = np.ascontiguousarray(x[b, g0:g0 + SEQ, :].T.astype(bf))
        m = np.full((128, 1), float(half), np.float32)
        in_maps.append(dict(xT=xTc, mask=m, **shared))
    return in_maps


def run_cores(inputs, debug_outputs=False, trace=False):
    nc = _build_program(debug_outputs=debug_outputs)
    in_maps = _host_prep(inputs)
    res = run_bass_kernel_spmd(nc, in_maps, list(range(NCORES)), trace=trace)
    return res


def kernel(**inputs) -> np.ndarray:
    res = run_cores(inputs)
    out = np.empty((B, N, D), np.float32)
    for c in range(NCORES):
        b, half = c // 2, c % 2
        out[b, half * SEQ:(half + 1) * SEQ, :] = res.results[c]["outT"].T
    return out


# revision 11
# speedup vs baseline: 1.0609x; 1.0542x over previous
"""Trainium2 Bass kernel for CausalWaveletFieldAttention (v2).

Shapes (hardcoded): x [B=4, N=4096, D=1024], H=16 heads, HD=64.
Sharding over 8 cores: core c handles (batch b = c//2, half = c%2), i.e.
2048 contiguous sequence rows of one batch.

Key ideas vs v1:
  * Permuted channel layout (d_sub, head): channel c_new = d*16 + h.
    Every 128-partition chunk then contains all 16 heads for 8 d-values,
    so the HxH head coupling becomes a block-diagonal [128,128] matmul
    -- and is FUSED into the PE conv-tap stationaries (coup @ diag(w_s))
    for free. The dense 1024x1024 coupling matmul is gone.
  * Conv computes only SEQ outputs (no 1024-col halo recompute); the
    skip taps' history (coupled field tail) comes from the pair peer via
    a second small AllGather.
  * Out projection in transposed layout (Wout chunks stationary, pg
    moving, output [D, SEQ]); host transposes when assembling. No pg
    DRAM round-trip, no rearrange DMAs.
  * bf16 for x / Wk / Wv / Wgate / Wout / gate / pg (PE rate unchanged,
    halves DMA + SBUF). f0 / conv accumulation stay fp32.
  * Conv taps split across PE (3 big shifts, coupling fused) and DVE
    (21 shifts as scalar_tensor_tensor chain); skips + masks + gate-mul
    on GpSimd. Gate projection strips interleaved into the conv phase
    to fill the PE.
"""

import numpy as np
import ml_dtypes

import concourse.bass as bass
import concourse.mybir as mybir
import concourse.tile as tile
from concourse import bacc
from concourse.bass_utils import run_bass_kernel_spmd

F32 = mybir.dt.float32
F32R = mybir.dt.float32r
BF16 = mybir.dt.bfloat16
AF = mybir.ActivationFunctionType
MUL = mybir.AluOpType.mult
ADD = mybir.AluOpType.add

B, N, D, H, HD = 4, 4096, 1024, 16, 64
NCORES = 8
SEQ = N // 2          # 2048 rows per core
KC = D // 128         # 8 contraction chunks
EXT2 = 2 * SEQ        # [peer 2048 | own 2048] conv input window
D4 = [0.4829629131445341, 0.8365163037378079, 0.2241438680420134, -0.1294095225512604]
N_SCALES = 11
SPARSE_DILATIONS = (512, 1024)
SHIFTS = [0, 1, 2, 3, 4, 6, 8, 12, 16, 24, 32, 48, 64, 96, 128, 192, 256,
          384, 512, 768, 1024, 1536, 2048, 3072]
NT = len(SHIFTS)      # 24 taps, all on DVE as G interleaved chains
G = 4                 # independent partial accumulators

_PROGRAM_CACHE = {}


def _build_program(debug_outputs=False):
    key = bool(debug_outputs)
    if key in _PROGRAM_CACHE:
        return _PROGRAM_CACHE[key]

    nc = bacc.Bacc("TRN2", target_bir_lowering=False, debug=False,
                   num_devices=NCORES)

    # ---- parameters (per-core) ----
    xT = nc.declare_dram_parameter("xT", [D, SEQ], BF16, isOutput=False)
    Wkv = nc.declare_dram_parameter("Wkv", [D, 2 * D], BF16, isOutput=False)
    Wg = nc.declare_dram_parameter("Wg", [D, D], BF16, isOutput=False)
    Wo = nc.declare_dram_parameter("Wo", [D, D], BF16, isOutput=False)
    bkvT = nc.declare_dram_parameter("bkvT", [128, 16], F32, isOutput=False)
    bgT = nc.declare_dram_parameter("bgT", [128, 8], F32, isOutput=False)
    boutT = nc.declare_dram_parameter("boutT", [128, 8], F32, isOutput=False)
    BDin = nc.declare_dram_parameter("BDin", [128, 128], F32R, isOutput=False)
    wtap = nc.declare_dram_parameter("wtap", [128, NT], F32, isOutput=False)
    swt = nc.declare_dram_parameter("swt", [128, 2], F32, isOutput=False)
    mask = nc.declare_dram_parameter("mask", [128, 1], F32, isOutput=False)
    ones16i = nc.declare_dram_parameter("ones16i", [128, 16], F32R,
                                        isOutput=False)
    on16i = nc.declare_dram_parameter("on16i", [16, 128], F32R, isOutput=False)
    outT = nc.declare_dram_parameter("outT", [D, SEQ], F32, isOutput=True)

    dbg = {}
    if debug_outputs:
        for name, shape, dt in (("dbg_f0", [D, SEQ], F32R),
                                ("dbg_field", [D, SEQ], F32),
                                ("dbg_pg", [D, SEQ], BF16)):
            dbg[name] = nc.declare_dram_parameter(name, shape, dt, isOutput=True)

    # ---- internal DRAM ----
    f0_dram = [nc.dram_tensor(f"f0_dram{c}", [128, SEQ], BF16)
               for c in range(KC)]
    f0_gath = [nc.dram_tensor(f"f0_gath{c}", [2, 128, SEQ], BF16)
               for c in range(KC)]
    tl_dram = [nc.dram_tensor(f"tl_dram{c}", [128, 1024], F32)
               for c in range(KC)]
    tl_gath = [nc.dram_tensor(f"tl_gath{c}", [2, 128, 1024], F32)
               for c in range(KC)]

    with tile.TileContext(nc) as tc:
        with (
            tc.tile_pool(name="psp", bufs=4, space="PSUM") as psp,
            tc.tile_pool(name="const", bufs=1) as constp,
            tc.tile_pool(name="p_x", bufs=1) as p_x,
            tc.tile_pool(name="p_gate", bufs=1) as p_gate,
            tc.tile_pool(name="p_pg", bufs=1) as p_pg,
            tc.tile_pool(name="p_strip", bufs=2) as p_strip,
        ):
            # ---- constants ----
            BD_t = constp.tile([128, 128], F32R)
            nc.sync.dma_start(BD_t[:], BDin[:])
            wtap_t = constp.tile([128, NT], F32)
            nc.sync.dma_start(wtap_t[:], wtap[:])
            swt_t = constp.tile([128, 2], F32)
            nc.sync.dma_start(swt_t[:], swt[:])
            mask_t = constp.tile([128, 1], F32)
            nc.sync.dma_start(mask_t[:], mask[:])
            bkv_t = constp.tile([128, 16], F32)
            nc.sync.dma_start(bkv_t[:], bkvT[:])
            bg_t = constp.tile([128, 8], F32)
            nc.sync.dma_start(bg_t[:], bgT[:])
            bout_t = constp.tile([128, 8], F32)
            nc.sync.dma_start(bout_t[:], boutT[:])
            ones16 = constp.tile([128, 16], F32R)
            nc.sync.dma_start(ones16[:], ones16i[:])
            on16 = constp.tile([16, 128], F32R)
            nc.sync.dma_start(on16[:], on16i[:])

            # ---- persistent big tiles ----
            xm = p_x.tile([128, KC, SEQ], BF16)
            for k in range(KC):
                nc.sync.dma_start(xm[:, k, :], xT[k * 128:(k + 1) * 128, :])
            gate = p_gate.tile([128, KC, SEQ], BF16)
            pg = p_pg.tile([128, KC, SEQ], BF16)

            def load_strip(src, col0):
                sr = p_strip.tile([128, KC, 128], BF16, tag="strip")
                nc.sync.dma_start(
                    sr[:],
                    src[:, col0 * 128:(col0 + 1) * 128]
                    .rearrange("(kc p) m -> p kc m", p=128))
                return sr

            def proj4(strip, tag="ps"):
                """4 PSUM tiles [128,512] = strip.T @ xm (contraction D)."""
                pss = [psp.tile([128, 512], F32, tag=tag, name=f"pj{rb}")
                       for rb in range(4)]
                for k in range(KC):
                    for rb in range(4):
                        nc.tensor.matmul(pss[rb][:], strip[:, k, :],
                                         xm[:, k, rb * 512:(rb + 1) * 512],
                                         start=(k == 0), stop=(k == KC - 1))
                return pss

            # ================= phase A: k -> k_mag =================
            with tc.tile_pool(name="p_km", bufs=1) as p_km:
              km_t = p_km.tile([16, SEQ], F32R)
              with (
                tc.tile_pool(name="pskm", bufs=1, space="PSUM") as pskm,
                tc.tile_pool(name="p_kv", bufs=2) as p_kv,
              ):
                km_pss = [pskm.tile([16, 512], F32, tag=f"km{sb}",
                                    name=f"km_ps{sb}")
                          for sb in range(4)]
                for c in range(KC):
                    ks = load_strip(Wkv, c)
                    pss = proj4(ks)
                    k2b = p_kv.tile([128, SEQ], F32R, tag="k2b")
                    for rb in range(4):
                        nc.scalar.activation(k2b[:, rb * 512:(rb + 1) * 512],
                                             pss[rb][:], AF.Square,
                                             bias=bkv_t[:, c:c + 1])
                    for sb in range(4):
                        nc.tensor.matmul(km_pss[sb][:], ones16[:],
                                         k2b[:, sb * 512:(sb + 1) * 512],
                                         start=(c == 0), stop=(c == KC - 1))
                for sb in range(4):
                    nc.scalar.activation(km_t[:, sb * 512:(sb + 1) * 512],
                                         km_pss[sb][:], AF.Sqrt)

              # =============== phase B: v -> f0 -> AllGather ===============
              with (
                tc.tile_pool(name="psv", bufs=2, space="PSUM") as psv,
                tc.tile_pool(name="p_v", bufs=2) as p_v,
              ):
                for c in range(KC):
                    vs = load_strip(Wkv, 8 + c)
                    pss = proj4(vs)
                    vTb = p_v.tile([128, SEQ], F32, tag="vTb")
                    for rb in range(4):
                        nc.scalar.activation(vTb[:, rb * 512:(rb + 1) * 512],
                                             pss[rb][:], AF.Identity,
                                             bias=bkv_t[:, 8 + c:9 + c])
                    f0b = p_v.tile([128, SEQ], BF16, tag="f0b")
                    for sb in range(4):
                        bps = psv.tile([128, 512], F32, tag="bc")
                        nc.tensor.matmul(bps[:], on16[:],
                                         km_t[:, sb * 512:(sb + 1) * 512],
                                         start=True, stop=True)
                        nc.vector.tensor_mul(f0b[:, sb * 512:(sb + 1) * 512],
                                             vTb[:, sb * 512:(sb + 1) * 512],
                                             bps[:])
                    nc.sync.dma_start(f0_dram[c][:], f0b[:])
                    nc.gpsimd.collective_compute(
                        "AllGather", mybir.AluOpType.bypass,
                        replica_groups=[[0, 1], [2, 3], [4, 5], [6, 7]],
                        ins=[f0_dram[c][:]], outs=[f0_gath[c][:]])

            if debug_outputs:
                for c in range(KC):
                    nc.sync.dma_start(
                        dbg["dbg_f0"][c * 128:(c + 1) * 128, :], f0_dram[c][:])

            # ======== phase C: conv + coupling + skips + gate (interleaved) ==
            with (
                tc.tile_pool(name="psc", bufs=3, space="PSUM") as psc,
                tc.tile_pool(name="p_ext", bufs=2) as p_ext,
                tc.tile_pool(name="p_acc", bufs=1) as p_acc,
                tc.tile_pool(name="p_fx", bufs=3) as p_fx,
            ):
                fexts = [None] * KC
                LAG = 2
                for c in range(KC + LAG):
                    if c < KC:
                        # conv input window [peer | own]
                        ext = p_ext.tile([128, EXT2], BF16, tag="ext")
                        nc.sync.dma_start(ext[:, 0:SEQ], f0_gath[c][0, :, :])
                        nc.scalar.activation(ext[:, 0:SEQ], ext[:, 0:SEQ],
                                             AF.Identity,
                                             scale=mask_t[:, 0:1])
                        nc.sync.dma_start(ext[:, SEQ:EXT2], f0_dram[c][:])

                        # all 24 taps on DVE as G interleaved partial chains
                        # (breaks the completion-semaphore latency chain)
                        parts = [p_acc.tile([128, SEQ], F32R, tag=f"pa{g}",
                                            name=f"pa{g}")
                                 for g in range(G)]
                        inited = [False] * G
                        for ti, s in enumerate(SHIFTS):
                            g = ti % G
                            j0 = max(0, s - SEQ)
                            src = ext[:, SEQ + j0 - s:EXT2 - s]
                            w = wtap_t[:, ti:ti + 1]
                            dst = parts[g][:, j0:SEQ]
                            if not inited[g]:
                                nc.vector.tensor_scalar_mul(dst, src, w)
                                inited[g] = True
                            else:
                                nc.vector.scalar_tensor_tensor(
                                    dst, src, w, dst, op0=MUL, op1=ADD)

                        # blockdiag coupling of the G partials -> field
                        fext = p_fx.tile([128, 1024 + SEQ], F32, tag="fext")
                        fexts[c] = fext
                        for ob in range(4):
                            ps = psc.tile([128, 512], F32, tag="cps")
                            for g in range(G):
                                nc.tensor.matmul(
                                    ps[:], BD_t[:],
                                    parts[g][:, ob * 512:(ob + 1) * 512],
                                    start=(g == 0), stop=(g == G - 1))
                            nc.scalar.activation(
                                fext[:, 1024 + ob * 512:1024 + (ob + 1) * 512],
                                ps[:], AF.Identity)

                        # coupled-field tail -> peer
                        nc.sync.dma_start(tl_dram[c][:], fext[:, SEQ:SEQ + 1024])
                        nc.gpsimd.collective_compute(
                            "AllGather", mybir.AluOpType.bypass,
                            replica_groups=[[0, 1], [2, 3], [4, 5], [6, 7]],
                            ins=[tl_dram[c][:]], outs=[tl_gath[c][:]])

                        # gate strip c (PE filler behind conv)
                        gs = load_strip(Wg, c)
                        pss = proj4(gs)
                        for rb in range(4):
                            nc.scalar.activation(
                                gate[:, c, rb * 512:(rb + 1) * 512],
                                pss[rb][:], AF.Sigmoid, bias=bg_t[:, c:c + 1])

                    if c >= LAG:
                        # skip taps + gate multiply for chunk c-LAG (peer tail
                        # has had two chunk-times to arrive)
                        cp = c - LAG
                        fext = fexts[cp]
                        nc.sync.dma_start(fext[:, 0:1024], tl_gath[cp][0, :, :])
                        nc.scalar.activation(fext[:, 0:1024], fext[:, 0:1024],
                                             AF.Identity,
                                             scale=mask_t[:, 0:1])
                        nc.vector.scalar_tensor_tensor(
                            pg[:, cp, :], fext[:, 512:512 + SEQ], swt_t[:, 0:1],
                            fext[:, 1024:1024 + SEQ], op0=MUL, op1=ADD)
                        nc.vector.scalar_tensor_tensor(
                            pg[:, cp, :], fext[:, 0:SEQ], swt_t[:, 1:2],
                            pg[:, cp, :], op0=MUL, op1=ADD)
                        nc.gpsimd.tensor_mul(pg[:, cp, :], pg[:, cp, :],
                                             gate[:, cp, :])
                        if debug_outputs:
                            nc.sync.dma_start(
                                dbg["dbg_field"][cp * 128:(cp + 1) * 128, :],
                                fext[:, 1024:1024 + SEQ])
                            nc.sync.dma_start(
                                dbg["dbg_pg"][cp * 128:(cp + 1) * 128, :],
                                pg[:, cp, :])

            # ================= phase D: outT = Wo.T-chunks @ pg =============
            with tc.tile_pool(name="p_out", bufs=2) as p_out:
                for co in range(KC):
                    wos = load_strip(Wo, co)
                    pss = [psp.tile([128, 512], F32, tag="ps",
                                    name=f"ops{sb}")
                           for sb in range(4)]
                    for k in range(KC):
                        for sb in range(4):
                            nc.tensor.matmul(pss[sb][:], wos[:, k, :],
                                             pg[:, k, sb * 512:(sb + 1) * 512],
                                             start=(k == 0), stop=(k == KC - 1))
                    outb = p_out.tile([128, SEQ], F32, tag="outb")
                    for sb in range(4):
                        nc.scalar.activation(outb[:, sb * 512:(sb + 1) * 512],
                                             pss[sb][:], AF.Identity,
                                             bias=bout_t[:, co:co + 1])
                    nc.sync.dma_start(outT[co * 128:(co + 1) * 128, :],
                                      outb[:])

    nc.compile()
    _PROGRAM_CACHE[key] = nc
    return nc


def _softmax(a, axis):
    a = a - a.max(axis=axis, keepdims=True)
    e = np.exp(a)
    return e / e.sum(axis=axis, keepdims=True)


def _blockdiag8(blk16):
    m = np.zeros((128, 128), np.float64)
    for d8 in range(8):
        m[d8 * 16:(d8 + 1) * 16, d8 * 16:(d8 + 1) * 16] = blk16
    return m


def _host_prep(inputs):
    """Build per-core and replicated input tensors from full inputs."""
    x = np.asarray(inputs["x"], np.float32)
    Wqkv = np.asarray(inputs["Wqkv"], np.float64)
    bqkv = np.asarray(inputs["bqkv"], np.float64)
    Wout = np.asarray(inputs["Wout"], np.float64)
    bout = np.asarray(inputs["bout"], np.float32)
    Wgate = np.asarray(inputs["Wgate"], np.float64)
    bgate = np.asarray(inputs["bgate"], np.float64)
    scale_gain = np.asarray(inputs["scale_gain"], np.float64)
    skip_w = np.asarray(inputs["skip_w"], np.float64)
    coupling = np.asarray(inputs["coupling"], np.float64)

    gains = _softmax(scale_gain, axis=0)              # [11, H]
    sw = 1.0 / (1.0 + np.exp(-skip_w))                # [2]
    coup = _softmax(coupling, axis=-1)                # [H, H]

    # channel permutation: new channel c_new = d*16 + h  <->  ref h*64 + d
    cn = np.arange(D)
    perm_idx = (cn % H) * HD + cn // H                # ref index per new chan

    sidx = {s: i for i, s in enumerate(SHIFTS)}
    wtab = np.zeros((len(SHIFTS), H), np.float64)
    for j in range(N_SCALES):
        d = 1 << j
        for t in range(4):
            wtab[sidx[(3 - t) * d]] += D4[t] * gains[j]

    # blockdiag coupling stationary: block[j, i] = coup[i, j]
    BDt = _blockdiag8(coup.T).astype(np.float32)

    wtap_np = np.zeros((128, NT), np.float32)
    for ti, s in enumerate(SHIFTS):
        wtap_np[:, ti] = wtab[sidx[s]][np.arange(128) % H]

    ones16 = np.zeros((128, 16), np.float32)
    ones16[np.arange(128), np.arange(128) % 16] = 1.0
    on16 = np.ascontiguousarray(ones16.T)             # [16, 128]

    # fold q projection into the gate
    Wq = Wqkv[:, :D]
    Wqg = Wq @ Wgate
    bg_f = bqkv[:D] @ Wgate + bgate

    bf = ml_dtypes.bfloat16
    Wk_p = Wqkv[:, D:2 * D][:, perm_idx]
    Wv_p = Wqkv[:, 2 * D:3 * D][:, perm_idx]
    Wkv_np = np.ascontiguousarray(
        np.concatenate([Wk_p, Wv_p], axis=1).astype(bf))
    Wg_np = np.ascontiguousarray(Wqg[:, perm_idx].astype(bf))
    Wo_np = np.ascontiguousarray(Wout[perm_idx, :].astype(bf))

    bk_p = bqkv[D:2 * D][perm_idx].astype(np.float32)
    bv_p = bqkv[2 * D:3 * D][perm_idx].astype(np.float32)
    bkvT_np = np.concatenate([bk_p.reshape(8, 128).T,
                              bv_p.reshape(8, 128).T], axis=1).copy()
    bgT_np = bg_f[perm_idx].astype(np.float32).reshape(8, 128).T.copy()
    boutT_np = bout.reshape(8, 128).T.copy()
    swt_np = np.broadcast_to(sw.astype(np.float32), (128, 2)).copy()

    shared = dict(Wkv=Wkv_np, Wg=Wg_np, Wo=Wo_np, bkvT=bkvT_np, bgT=bgT_np,
                  boutT=boutT_np, BDin=BDt, wtap=wtap_np, swt=swt_np,
                  ones16i=ones16, on16i=on16)
    in_maps = []
    for c in range(NCORES):
        b, half = c // 2, c % 2
        g0 = half * SEQ
        xTc = np.ascontiguousarray(x[b, g0:g0 + SEQ, :].T.astype(bf))
        m = np.full((128, 1), float(half), np.float32)
        in_maps.append(dict(xT=xTc, mask=m, **shared))
    return in_maps


def run_cores(inputs, debug_outputs=False, trace=False):
    nc = _build_program(debug_outputs=debug_outputs)
    in_maps = _host_prep(inputs)
    res = run_bass_kernel_spmd(nc, in_maps, list(range(NCORES)), trace=trace)
    return res


def kernel(**inputs) -> np.ndarray:
    res = run_cores(inputs)
    out = np.empty((B, N, D), np.float32)
    for c in range(NCORES):
        b, half = c // 2, c % 2
        out[b, half * SEQ:(half + 1) * SEQ, :] = res.results[c]["outT"].T
    return out
